# revision 14
# baseline (speedup 1.0000x reference)
"""Trainium2 Bass kernel for AfmoeMoE: token-choice top-2 MoE + shared expert.

Sharding (8 cores):
  - Routed experts: expert-parallel. Core c owns expert c's GLU-MLP weights;
    the host routes tokens (router math replicated bit-exactly on CPU jax),
    gathers each expert's tokens into a fixed-capacity buffer (the
    "all-to-all dispatch"), and scatter-adds results back with the routing
    weight applied host-side.
  - Shared expert: tensor-parallel over FS in halves x data-parallel over
    4 token groups. Core c handles token group c//2 with FS-half c%2;
    the two halves' partial outputs are summed on the host.

Per-core device kernel (~58.8k PE cycles/body = the fp16 compute roofline;
fp8 DoubleRow would halve cycles but measures 3-5e-2 rel err vs the 2e-2
gate, even routed-only):
  - Matmul operands (x, weights, h) are fp16 (e5m10): full PE rate and half
    the HBM bytes of fp32, at ~6e-4 output error. PSUM accumulation stays
    fp32; DRAM outputs are fp16 (host upcasts). MM_MODE switches dtypes.
  - Weights (7.1 MB/core fp16) are RESIDENT in SBUF: loaded once per
    program before the repeat loop; each body streams only x in (1.1 MB,
    SP HWDGE ring) and outputs out (1.1 MB, GPSIMD SWDGE), so steady state
    is PE-bound, not HBM-bound.
  - Stage 1: m-outer G pass (bank m's accumulation group closes right
    after its own k-loop, so silu(G_m) overlaps the remaining G matmuls;
    k-outer closed every bank only at pass end, which stalled the first U
    matmul ~745ns on ACT latency), then an m-outer U pass so bank m's U
    accumulation starts right after its own silu drains it.
  - Routed stage 2 is d-major: O[dt] = Wd[:,dt]^T @ h keeps tokens in the
    free dim (48 MMs x 280 free instead of 36 x 512), output [D, cap] fp16
    unscaled; the host transposes and applies the routing weight. Shared
    stage 2 stays token-major ([256, D] needs no per-token scale).
  - PSUM: 6 banks for G/U (tags GU0-5, shared expert reuses GU0-2) + 2
    for stage-2 accumulation (tag O). The repeat loop unrolls 4 bodies per
    hardware-loop iteration (measured ~2us/iter better than unroll=2).
"""

import math

import numpy as np

B, S, D = 2, 512, 1024
T = B * S
E = 8
F = 768
FS = 768
TOP_K = 2
EPS = 1e-20
ROUTE_SCALE = 1.0
P = 128
N_CORES = 8
SGRP = 256        # shared-expert tokens per core (4 groups x 2 FS-halves)
FSH = FS // 2     # shared-expert intermediate slice per core
PAIR = 512        # token-group size for stage-1 matmuls (rhs free dim)

_compiled = {}
MM_MODE = "f16"


def round_f32r(x):
    """Round fp32 to the PE's fp32r format: e8m11, low 12 mantissa bits zero (RNE)."""
    u = np.ascontiguousarray(x, np.float32).view(np.uint32)
    lsb = (u >> 12) & 1
    u2 = (u + 0x7FF + lsb) & np.uint32(0xFFFFF000)
    return u2.view(np.float32)


def build_nc(cap, repeat=1, act="silu", mm=None, flat=False, unroll=None,
             xt_bufs=2, h_bufs=3, otr_bufs=2, ot_bufs=4, out_eng="gpsimd",
             g_order="m"):
    if mm is None:
        mm = MM_MODE
    """Build the per-core Bass program (same program on all 8 cores)."""
    import concourse.bacc as bacc
    import concourse.mybir as mybir
    import concourse.tile as tile

    f32 = mybir.dt.float32
    wdt = {"f32r": mybir.dt.float32r, "bf16": mybir.dt.bfloat16,
           "f16": mybir.dt.float16, "f32": mybir.dt.float32}[mm]
    silu = mybir.ActivationFunctionType.Silu
    sigmoid = mybir.ActivationFunctionType.Sigmoid
    KD = D // P    # 8 contraction chunks
    MF = F // P    # 6 expert F-tiles
    MS = FSH // P  # 3 shared F-tiles
    NT = (cap + P - 1) // P  # routed token tiles

    f16 = mybir.dt.float16
    nc = bacc.Bacc("TRN2", target_bir_lowering=False, debug=False,
                   num_devices=N_CORES)

    x_in = nc.dram_tensor("x_in", [D, cap + SGRP], wdt, kind="ExternalInput")
    wgu_e = nc.dram_tensor("wgu_e", [D, 2 * F], wdt, kind="ExternalInput")
    wd_e = nc.dram_tensor("wd_e", [F, D], wdt, kind="ExternalInput")
    wgu_s = nc.dram_tensor("wgu_s", [D, 2 * FSH], wdt, kind="ExternalInput")
    wd_s = nc.dram_tensor("wd_s", [FSH, D], wdt, kind="ExternalInput")
    # routed output is [D, cap] (d-major, unscaled): stage 2 keeps tokens in
    # the free dim, the host applies the routing weight + transpose.
    r_out = nc.dram_tensor("r_out", [D, cap], f16, kind="ExternalOutput")
    s_out = nc.dram_tensor("s_out", [SGRP, D], f16, kind="ExternalOutput")
    oeng = {"gpsimd": nc.gpsimd, "scalar": nc.scalar, "vector": nc.vector}[out_eng]

    with tile.TileContext(nc) as tc:
        with (
            tc.tile_pool(name="wp", bufs=1) as wp,
            tc.tile_pool(name="dp", bufs=3) as dp,
            tc.tile_pool(name="pp", bufs=2, space="PSUM") as pp,
        ):
            def g_pass(xt, x_off, ntk, wgu, fstride, mf):
                """m-outer G-pass: bank m's accumulation group closes after its
                own k-loop, so silu(G_m) (and then U_m) starts while G_{m+1}..
                still run on PE (k-outer closed every bank only at pass end,
                stalling the first U matmul ~745ns on ACT latency)."""
                xs = xt[:, :, x_off: x_off + ntk]
                Gs = [pp.tile([P, PAIR], f32, name=f"GU{m}", tag=f"GU{m}", bufs=1)[:, :ntk]
                      for m in range(mf)]
                loops = ([(m, kc) for m in range(mf) for kc in range(KD)]
                         if g_order == "m" else
                         [(m, kc) for kc in range(KD) for m in range(mf)])
                for m, kc in loops:
                    nc.tensor.matmul(Gs[m], wgu[:, kc, m * P:(m + 1) * P],
                                     xs[:, kc, :],
                                     start=(kc == 0), stop=(kc == KD - 1))
                return Gs

            def u_pass(xt, x_off, ntk, wgu, fstride, mf, Gs):
                """silu(G) -> h, then U-pass reusing the G banks, h *= U.

                m-outer so bank m's U accumulation starts right after its own
                silu drains it (k-outer would gate U's first k-sweep on all mf
                silus serially on ACT)."""
                xs = xt[:, :, x_off: x_off + ntk]
                h = dp.tile([P, mf, PAIR], wdt, name="h", tag="h", bufs=h_bufs)
                for m in range(mf):
                    if act == "silu":
                        nc.scalar.activation(h[:, m, :ntk], Gs[m], silu)
                    else:
                        # CoreSim lacks Silu: silu(G) = G * sigmoid(G)
                        nc.scalar.activation(h[:, m, :ntk], Gs[m], sigmoid)
                        nc.vector.tensor_mul(h[:, m, :ntk], h[:, m, :ntk], Gs[m])
                    U = pp.tile([P, PAIR], f32, name=f"GU{m}b", tag=f"GU{m}", bufs=1)[:, :ntk]
                    for kc in range(KD):
                        nc.tensor.matmul(U,
                                         wgu[:, kc, fstride + m * P:fstride + (m + 1) * P],
                                         xs[:, kc, :],
                                         start=(kc == 0), stop=(kc == KD - 1))
                    nc.vector.tensor_mul(h[:, m, :ntk], h[:, m, :ntk], U)
                return h

            def stage2(h, ntk, tt0, wdt_sb, mf, out_dram):
                """out[tile] = h_tile^T @ Wd -> DRAM, per 128-token tile."""
                ntiles = (ntk + P - 1) // P
                for tp in range(ntiles):
                    th = min(P, ntk - tp * P)
                    tok0 = tp * P
                    tt = tt0 + tp
                    ot = dp.tile([P, D], f16, name="ot", tag="ot", bufs=ot_bufs)
                    for di in range(D // 512):
                        O = pp.tile([P, 512], f32, name="O", tag="O", bufs=2)
                        for m in range(mf):
                            nc.tensor.matmul(O[:th],
                                             h[:, m, tok0:tok0 + th],
                                             wdt_sb[:, m, di * 512:(di + 1) * 512],
                                             start=(m == 0), stop=(m == mf - 1))
                        nc.vector.tensor_copy(ot[:th, di * 512:(di + 1) * 512], O[:th])
                    oeng.dma_start(out=out_dram[tt * P: tt * P + th, :], in_=ot[:th])

            def stage2_r(h, ntk, wdt_sb, mf, out_dram):
                """Routed stage 2, d-major: O[dt] = Wd[:, dt]^T @ h  (tokens stay
                in the free dim, so 128-d tiles replace 512-wide moving ops and
                the per-token routing weight is applied on the host)."""
                KDT = D // P  # 8 output d-tiles
                osrc = out_dram.ap().rearrange("(dp p) t -> p dp t", p=P)
                ot = dp.tile([P, KDT, ntk], f16, name="otr", tag="otr", bufs=otr_bufs)
                for dt in range(KDT):
                    O = pp.tile([P, 512], f32, name="O", tag="O", bufs=2)[:, :ntk]
                    for m in range(mf):
                        nc.tensor.matmul(O,
                                         wdt_sb[:, m, dt * P:(dt + 1) * P],
                                         h[:, m, :ntk],
                                         start=(m == 0), stop=(m == mf - 1))
                    nc.vector.tensor_copy(ot[:, dt], O)
                    if dt % 4 == 3:
                        oeng.dma_start(out=osrc[:, dt - 3:dt + 1],
                                       in_=ot[:, dt - 3:dt + 1])

            def load_chunked(eng, dram, shape, rearr, name, nchunks):
                """SBUF tile filled by per-chunk DMAs (lets compute start early)."""
                t = wp.tile([P, *shape], wdt, name=name)
                src = dram.ap().rearrange(rearr, p=P)
                if nchunks == 1:
                    eng.dma_start(out=t[:], in_=src[:])
                else:
                    step = shape[0] // nchunks
                    for i in range(nchunks):
                        eng.dma_start(out=t[:, i * step:(i + 1) * step],
                                      in_=src[:, i * step:(i + 1) * step])
                return t

            def weight_dmas():
                # Weights load ONCE per program (resident in SBUF across the
                # repeat loop); ordered by first-body consumption time.
                wgu_sb = wp.tile([P, KD, 2 * F], wdt, name="wgu_sb")
                wgu_src = wgu_e.ap().rearrange("(k p) f -> p k f", p=P)
                for lo in range(0, KD, 2):
                    nc.sync.dma_start(out=wgu_sb[:, lo:lo + 2, :F],
                                      in_=wgu_src[:, lo:lo + 2, :F])
                for lo in range(0, KD, 4):
                    nc.sync.dma_start(out=wgu_sb[:, lo:lo + 4, F:], in_=wgu_src[:, lo:lo + 4, F:])
                wd_sb = load_chunked(nc.sync, wd_e, [MF, D], "(m p) d -> p m d", "wd_sb", 3)
                wgus_sb = load_chunked(nc.sync, wgu_s, [KD, 2 * FSH], "(k p) f -> p k f", "wgus_sb", 2)
                wds_sb = load_chunked(nc.sync, wd_s, [MS, D], "(m p) d -> p m d", "wds_sb", 1)
                return wgu_sb, wd_sb, wgus_sb, wds_sb

            def input_dmas():
                # Per-body activation stream (SP ring).
                xt = dp.tile([P, KD, cap + SGRP], wdt, name="xt", tag="xt", bufs=xt_bufs)
                xsrc = x_in.ap().rearrange("(k p) n -> p k n", p=P)
                for lo in range(0, KD, 2):
                    nc.sync.dma_start(out=xt[:, lo:lo + 2], in_=xsrc[:, lo:lo + 2])
                return xt

            def body(weights, staged=False):
                wgu_sb, wd_sb, wgus_sb, wds_sb = weights
                xt = input_dmas()
                # routed: single stage-1 group when cap <= PAIR (always true for
                # this input); fall back to a grouped loop otherwise.
                if cap <= PAIR:
                    Gs = g_pass(xt, 0, cap, wgu_sb, F, MF)
                    if staged:
                        tc.stage_boundary()
                    h_r = u_pass(xt, 0, cap, wgu_sb, F, MF, Gs)
                    if staged:
                        tc.stage_boundary()
                    stage2_r(h_r, cap, wd_sb, MF, r_out)
                    if staged:
                        tc.stage_boundary()
                else:
                    raise NotImplementedError("cap > PAIR not supported")
                Gs_s = g_pass(xt, cap, SGRP, wgus_sb, FSH, MS)
                h_s = u_pass(xt, cap, SGRP, wgus_sb, FSH, MS, Gs_s)
                stage2(h_s, SGRP, 0, wds_sb, MS, s_out)

            warm = dp.tile([P, 1], f32, name="warm", tag="warm", bufs=1)
            nc.vector.memset(warm[:], 0.0)
            nc.scalar.activation(warm[:], warm[:],
                                 silu if act == "silu" else sigmoid)
            weights = weight_dmas()

            if repeat == 1:
                body(weights)
            elif flat:
                # Python-unrolled (no hw loop): for TimelineSim gap analysis.
                for _ in range(repeat):
                    body(weights)
            else:
                # unroll 2 bodies per hardware-loop iteration: the Tile
                # scheduler freely pipelines adjacent invocations (stage
                # gates then only bite every other body)
                if unroll is None:
                    unroll = 4 if repeat % 4 == 0 else (2 if repeat % 2 == 0 else 1)
                with tc.For_i(0, repeat // unroll, 1,
                              hint_engines=(mybir.EngineType.PE,),
                              staggered_reset=True):
                    for _ in range(unroll):
                        body(weights, staged=False)

    nc.compile()
    return nc


def build_nc_bal(ns, ntot, repeat=1, act="silu", mm=None, flat=False, unroll=None,
                 xt_bufs=2, h_bufs=3, ot_bufs=2, out_eng="gpsimd"):
    """Balanced program: every core runs `ntot` columns of identical-cost
    work -- `ns` columns through its own expert's GLU weights plus
    `ntot - ns` shared-expert tokens (full-F shared weights, replicated).
    The split point `ns` is compile-time per core (8 program variants);
    all DRAM tensor shapes are identical across cores.

    Per-body PE cycles: 144 * ntot (ntot=384 -> 55296 = 23.04us floor),
    vs 144*cap + 18432 for the capacity-padded SPMD program."""
    if mm is None:
        mm = MM_MODE
    import concourse.bacc as bacc
    import concourse.mybir as mybir
    import concourse.tile as tile

    f32 = mybir.dt.float32
    wdt = {"f32r": mybir.dt.float32r, "bf16": mybir.dt.bfloat16,
           "f16": mybir.dt.float16, "f32": mybir.dt.float32}[mm]
    silu = mybir.ActivationFunctionType.Silu
    sigmoid = mybir.ActivationFunctionType.Sigmoid
    KD = D // P    # 8 contraction chunks
    MF = F // P    # 6 F-tiles (routed and shared both use full F=768)
    nsh = ntot - ns

    f16 = mybir.dt.float16
    nc = bacc.Bacc("TRN2", target_bir_lowering=False, debug=False,
                   num_devices=N_CORES)

    x_in = nc.dram_tensor("x_in", [D, ntot], wdt, kind="ExternalInput")
    wgu_e = nc.dram_tensor("wgu_e", [D, 2 * F], wdt, kind="ExternalInput")
    wd_e = nc.dram_tensor("wd_e", [F, D], wdt, kind="ExternalInput")
    wgu_s = nc.dram_tensor("wgu_s", [D, 2 * F], wdt, kind="ExternalInput")
    wd_s = nc.dram_tensor("wd_s", [F, D], wdt, kind="ExternalInput")
    # single d-major output [D, ntot] fp16, unscaled; host scales routed
    # columns by the routing weight and scatter-adds everything.
    o_out = nc.dram_tensor("o_out", [D, ntot], f16, kind="ExternalOutput")
    oeng = {"gpsimd": nc.gpsimd, "scalar": nc.scalar, "vector": nc.vector}[out_eng]

    with tile.TileContext(nc) as tc:
        with (
            tc.tile_pool(name="wp", bufs=1) as wp,
            tc.tile_pool(name="dp", bufs=3) as dp,
            tc.tile_pool(name="pp", bufs=2, space="PSUM") as pp,
        ):
            def gu_matmuls(dst, xt, we, ws, foff, m):
                """One bank's accumulation: expert part then shared part
                (two independent accumulation chains, disjoint columns)."""
                for kc in range(KD):
                    nc.tensor.matmul(dst[:, :ns],
                                     we[:, kc, foff + m * P:foff + (m + 1) * P],
                                     xt[:, kc, :ns],
                                     start=(kc == 0), stop=(kc == KD - 1))
                for kc in range(KD):
                    nc.tensor.matmul(dst[:, ns:],
                                     ws[:, kc, foff + m * P:foff + (m + 1) * P],
                                     xt[:, kc, ns:],
                                     start=(kc == 0), stop=(kc == KD - 1))

            def body(weights):
                wgu_e_sb, wd_e_sb, wgu_s_sb, wd_s_sb = weights
                xt = dp.tile([P, KD, ntot], wdt, name="xt", tag="xt", bufs=xt_bufs)
                xsrc = x_in.ap().rearrange("(k p) n -> p k n", p=P)
                for lo in range(0, KD, 2):
                    nc.sync.dma_start(out=xt[:, lo:lo + 2], in_=xsrc[:, lo:lo + 2])

                # ---- stage 1: G pass (m-outer), then silu+U+mul per bank ----
                Gs = [pp.tile([P, ntot], f32, name=f"GU{m}", tag=f"GU{m}", bufs=1)
                      for m in range(MF)]
                for m in range(MF):
                    gu_matmuls(Gs[m], xt, wgu_e_sb, wgu_s_sb, 0, m)
                h = dp.tile([P, MF, ntot], wdt, name="h", tag="h", bufs=h_bufs)
                for m in range(MF):
                    if act == "silu":
                        nc.scalar.activation(h[:, m, :], Gs[m], silu)
                    else:
                        nc.scalar.activation(h[:, m, :], Gs[m], sigmoid)
                        nc.vector.tensor_mul(h[:, m, :], h[:, m, :], Gs[m])
                    U = pp.tile([P, ntot], f32, name=f"GU{m}b", tag=f"GU{m}", bufs=1)
                    gu_matmuls(U, xt, wgu_e_sb, wgu_s_sb, F, m)
                    nc.vector.tensor_mul(h[:, m, :], h[:, m, :], U)

                # ---- stage 2: d-major, O[dt] = Wd[:, dt]^T @ h ----
                osrc = o_out.ap().rearrange("(dp p) t -> p dp t", p=P)
                ot = dp.tile([P, KD, ntot], f16, name="ot", tag="ot", bufs=ot_bufs)
                for dt in range(KD):
                    O = pp.tile([P, ntot], f32, name="O", tag="O", bufs=2)
                    for m in range(MF):
                        nc.tensor.matmul(O[:, :ns],
                                         wd_e_sb[:, m, dt * P:(dt + 1) * P],
                                         h[:, m, :ns],
                                         start=(m == 0), stop=(m == MF - 1))
                    for m in range(MF):
                        nc.tensor.matmul(O[:, ns:],
                                         wd_s_sb[:, m, dt * P:(dt + 1) * P],
                                         h[:, m, ns:],
                                         start=(m == 0), stop=(m == MF - 1))
                    nc.vector.tensor_copy(ot[:, dt], O)
                    if dt % 4 == 3:
                        oeng.dma_start(out=osrc[:, dt - 3:dt + 1],
                                       in_=ot[:, dt - 3:dt + 1])

            def load_w(dram, shape, rearr, name, nchunks):
                t = wp.tile([P, *shape], wdt, name=name)
                src = dram.ap().rearrange(rearr, p=P)
                step = shape[0] // nchunks
                for i in range(nchunks):
                    nc.sync.dma_start(out=t[:, i * step:(i + 1) * step],
                                      in_=src[:, i * step:(i + 1) * step])
                return t

            warm = dp.tile([P, 1], f32, name="warm", tag="warm", bufs=1)
            nc.vector.memset(warm[:], 0.0)
            nc.scalar.activation(warm[:], warm[:],
                                 silu if act == "silu" else sigmoid)
            weights = (
                load_w(wgu_e, [KD, 2 * F], "(k p) f -> p k f", "wgue_sb", 4),
                load_w(wd_e, [MF, D], "(m p) d -> p m d", "wde_sb", 3),
                load_w(wgu_s, [KD, 2 * F], "(k p) f -> p k f", "wgus_sb", 4),
                load_w(wd_s, [MF, D], "(m p) d -> p m d", "wds_sb", 3),
            )

            if repeat == 1:
                body(weights)
            elif flat:
                for _ in range(repeat):
                    body(weights)
            else:
                if unroll is None:
                    unroll = 4 if repeat % 4 == 0 else (2 if repeat % 2 == 0 else 1)
                with tc.For_i(0, repeat // unroll, 1,
                              hint_engines=(mybir.EngineType.PE,),
                              staggered_reset=True):
                    for _ in range(unroll):
                        body(weights)

    nc.compile()
    return nc


def _route(x, Wr, bias):
    """Replicate the reference router numerics (jax on CPU)."""
    import jax
    import jax.numpy as jnp

    cpu = jax.devices("cpu")[0]
    with jax.default_device(cpu):
        xj = jax.device_put(np.asarray(x, np.float32), cpu)
        Wj = jax.device_put(np.asarray(Wr, np.float32), cpu)
        bj = jax.device_put(np.asarray(bias, np.float32), cpu)
        logits = xj @ Wj
        scores = jax.nn.sigmoid(logits.astype(jnp.float32))
        _, sel = jax.lax.top_k(scores + bj, TOP_K)
        top = jnp.take_along_axis(scores, sel, axis=1)
        top = top / (top.sum(-1, keepdims=True) + EPS)
        top = top * ROUTE_SCALE
        return np.asarray(sel), np.asarray(top, np.float32)


def prepare(hidden_states, W_gate_router, expert_bias, Wg, Wu, Wd, Wg_s, Wu_s, Wd_s):
    """Host-side routing + sharding. Returns (cap, in_maps, combine_fn)."""
    x = np.ascontiguousarray(np.asarray(hidden_states, np.float32).reshape(T, D))
    sel, wts = _route(x, W_gate_router, expert_bias)

    tok = np.repeat(np.arange(T), TOP_K)
    expf = np.asarray(sel).reshape(-1)
    wf = np.asarray(wts).reshape(-1)
    counts = np.bincount(expf, minlength=E)
    cap = max(256, int(math.ceil(counts.max() / 8)) * 8)
    if cap > PAIR:
        # multi-group: keep 128-granularity and a trailing group of >= 256
        # tokens (N < 256 matmuls run at 1/4 rate in fp32r)
        cap = max(256, int(math.ceil(counts.max() / P)) * P)
        if cap % PAIR == P:
            cap += P

    order = np.argsort(expf, kind="stable")
    starts = np.zeros(E + 1, np.int64)
    starts[1:] = np.cumsum(counts)

    if MM_MODE == "f32r":
        rnd = round_f32r
    elif MM_MODE == "bf16":
        import ml_dtypes
        rnd = lambda a: np.ascontiguousarray(np.asarray(a, np.float32).astype(ml_dtypes.bfloat16))
    elif MM_MODE == "f16":
        rnd = lambda a: np.ascontiguousarray(np.asarray(a, np.float32).astype(np.float16))
    else:
        rnd = lambda a: np.ascontiguousarray(a, np.float32)
    in_maps = []
    toklists = []
    wlists = []
    Wg = np.asarray(Wg, np.float32)
    Wu = np.asarray(Wu, np.float32)
    Wd = np.asarray(Wd, np.float32)
    Wg_s = np.asarray(Wg_s, np.float32)
    Wu_s = np.asarray(Wu_s, np.float32)
    Wd_s = np.asarray(Wd_s, np.float32)
    xrnd = rnd(x)
    for c in range(N_CORES):
        g, hh = divmod(c, 2)
        sl = order[starts[c]:starts[c + 1]]
        n_c = counts[c]
        xr = np.zeros((cap, D), xrnd.dtype)
        xr[:n_c] = xrnd[tok[sl]]
        toklists.append(tok[sl])
        wlists.append(wf[sl].astype(np.float32))
        x_all = np.concatenate([xr, xrnd[g * SGRP:(g + 1) * SGRP]], axis=0)
        in_maps.append({
            "x_in": np.ascontiguousarray(x_all.T),
            "wgu_e": rnd(np.concatenate([Wg[c], Wu[c]], axis=1)),
            "wd_e": rnd(Wd[c]),
            "wgu_s": rnd(np.concatenate([Wg_s[:, hh * FSH:(hh + 1) * FSH],
                                         Wu_s[:, hh * FSH:(hh + 1) * FSH]], axis=1)),
            "wd_s": rnd(Wd_s[hh * FSH:(hh + 1) * FSH, :]),
        })

    def combine(results):
        out = np.zeros((T, D), np.float32)
        for c in range(N_CORES):
            g, hh = divmod(c, 2)
            out[g * SGRP:(g + 1) * SGRP] += results[c]["s_out"].astype(np.float32)
            n_c = counts[c]
            if n_c:
                rt = results[c]["r_out"][:, :n_c].astype(np.float32).T  # [n_c, D]
                out[toklists[c]] += rt * wlists[c][:, None]
        return out.reshape(B, S, D)

    return cap, in_maps, combine


def build_nc_pair(nA, nB, repeat=1, act="silu", mm=None, flat=False, unroll=None,
                  xt_bufs=3, h_bufs=3, ot_bufs=2, o_bufs=3, out_eng="gpsimd",
                  cp_eng="vector"):
    """Pair-F-split program (uniform SPMD): each core holds HALF the F dim
    (384 = 3 tiles) of TWO routed experts (a hot one, zone A = nA cols, and a
    cold one, zone B = nB cols) plus the usual shared-expert half (zone S =
    SGRP cols).  An expert's F-halves live on cores 2k / 2k+1; the host sums
    the pair's partial outputs.  Zones are sized to the max count within the
    hot/cold groups, so padding is (maxA-count)+(maxB-count) instead of
    (cap-count)*full-F: 72*(nA+nB+SGRP) = ~56.7k PE cycles vs 58.75k for the
    capacity design."""
    if mm is None:
        mm = MM_MODE
    import concourse.bacc as bacc
    import concourse.mybir as mybir
    import concourse.tile as tile

    f32 = mybir.dt.float32
    wdt = {"f32r": mybir.dt.float32r, "bf16": mybir.dt.bfloat16,
           "f16": mybir.dt.float16, "f32": mybir.dt.float32}[mm]
    silu = mybir.ActivationFunctionType.Silu
    sigmoid = mybir.ActivationFunctionType.Sigmoid
    KD = D // P     # 8 contraction chunks
    MH = FSH // P   # 3 F-tiles per half
    nS = SGRP
    ntot = nA + nB + nS
    zones = [(0, nA), (nA, nA + nB), (nA + nB, ntot)]

    f16 = mybir.dt.float16
    nc = bacc.Bacc("TRN2", target_bir_lowering=False, debug=False,
                   num_devices=N_CORES)

    x_in = nc.dram_tensor("x_in", [D, ntot], wdt, kind="ExternalInput")
    wgu_A = nc.dram_tensor("wgu_A", [D, 2 * FSH], wdt, kind="ExternalInput")
    wd_A = nc.dram_tensor("wd_A", [FSH, D], wdt, kind="ExternalInput")
    wgu_B = nc.dram_tensor("wgu_B", [D, 2 * FSH], wdt, kind="ExternalInput")
    wd_B = nc.dram_tensor("wd_B", [FSH, D], wdt, kind="ExternalInput")
    wgu_S = nc.dram_tensor("wgu_S", [D, 2 * FSH], wdt, kind="ExternalInput")
    wd_S = nc.dram_tensor("wd_S", [FSH, D], wdt, kind="ExternalInput")
    # single d-major half-F-partial output; host sums core pairs, scales
    # routed columns, scatter-adds.
    o_out = nc.dram_tensor("o_out", [D, ntot], f16, kind="ExternalOutput")
    oeng = {"gpsimd": nc.gpsimd, "scalar": nc.scalar, "vector": nc.vector}[out_eng]

    with tile.TileContext(nc) as tc:
        with (
            tc.tile_pool(name="wp", bufs=1) as wp,
            tc.tile_pool(name="dp", bufs=3) as dp,
            tc.tile_pool(name="pp", bufs=2, space="PSUM") as pp,
        ):
            def section(xt, h, z, wgu_sb, wd_sb, ot):
                c0, c1 = zones[z]
                ncols = c1 - c0
                xs = xt[:, :, c0:c1]
                # G pass, m-outer (banks close early -> silu overlaps)
                Gs = [pp.tile([P, 512], f32, name=f"GU{m}z{z}", tag=f"GU{m}",
                              bufs=1)[:, :ncols] for m in range(MH)]
                for m in range(MH):
                    for kc in range(KD):
                        nc.tensor.matmul(Gs[m], wgu_sb[:, kc, m * P:(m + 1) * P],
                                         xs[:, kc, :],
                                         start=(kc == 0), stop=(kc == KD - 1))
                # silu -> h, U pass into same banks, h *= U
                for m in range(MH):
                    if act == "silu":
                        nc.scalar.activation(h[:, m, c0:c1], Gs[m], silu)
                    else:
                        nc.scalar.activation(h[:, m, c0:c1], Gs[m], sigmoid)
                        nc.vector.tensor_mul(h[:, m, c0:c1], h[:, m, c0:c1], Gs[m])
                    U = pp.tile([P, 512], f32, name=f"GU{m}z{z}b", tag=f"GU{m}",
                                bufs=1)[:, :ncols]
                    for kc in range(KD):
                        nc.tensor.matmul(U,
                                         wgu_sb[:, kc, FSH + m * P:FSH + (m + 1) * P],
                                         xs[:, kc, :],
                                         start=(kc == 0), stop=(kc == KD - 1))
                    nc.vector.tensor_mul(h[:, m, c0:c1], h[:, m, c0:c1], U)
                # stage 2, d-major: O[dt] = Wd[:, dt]^T @ h_section
                osrc = o_out.ap().rearrange("(dp p) t -> p dp t", p=P)
                for dt in range(KD):
                    O = pp.tile([P, 512], f32, name=f"Oz{z}", tag="O",
                                bufs=o_bufs)[:, :ncols]
                    for m in range(MH):
                        nc.tensor.matmul(O,
                                         wd_sb[:, m, dt * P:(dt + 1) * P],
                                         h[:, m, c0:c1],
                                         start=(m == 0), stop=(m == MH - 1))
                    if cp_eng == "vector":
                        nc.vector.tensor_copy(ot[:, dt, c0:c1], O)
                    elif cp_eng == "gpsimd":
                        nc.gpsimd.tensor_copy(ot[:, dt, c0:c1], O)
                    elif cp_eng == "scalar":
                        nc.scalar.copy(ot[:, dt, c0:c1], O)
                    else:  # mixed: alternate DVE / Pool to split the load
                        if dt % 2 == 0:
                            nc.vector.tensor_copy(ot[:, dt, c0:c1], O)
                        else:
                            nc.gpsimd.tensor_copy(ot[:, dt, c0:c1], O)
                    if dt % 4 == 3:
                        oeng.dma_start(out=osrc[:, dt - 3:dt + 1, c0:c1],
                                       in_=ot[:, dt - 3:dt + 1, c0:c1])

            def body(weights):
                (wguA_sb, wdA_sb, wguB_sb, wdB_sb, wguS_sb, wdS_sb) = weights
                xt = dp.tile([P, KD, ntot], wdt, name="xt", tag="xt", bufs=xt_bufs)
                xsrc = x_in.ap().rearrange("(k p) n -> p k n", p=P)
                for lo in range(0, KD, 2):
                    nc.sync.dma_start(out=xt[:, lo:lo + 2], in_=xsrc[:, lo:lo + 2])
                h = dp.tile([P, MH, ntot], wdt, name="h", tag="h", bufs=h_bufs)
                ot = dp.tile([P, KD, ntot], f16, name="ot", tag="ot", bufs=ot_bufs)
                section(xt, h, 0, wguA_sb, wdA_sb, ot)
                section(xt, h, 1, wguB_sb, wdB_sb, ot)
                section(xt, h, 2, wguS_sb, wdS_sb, ot)

            def load_w(dram, shape, rearr, name, nchunks):
                t = wp.tile([P, *shape], wdt, name=name)
                src = dram.ap().rearrange(rearr, p=P)
                step = shape[0] // nchunks
                for i in range(nchunks):
                    nc.sync.dma_start(out=t[:, i * step:(i + 1) * step],
                                      in_=src[:, i * step:(i + 1) * step])
                return t

            warm = dp.tile([P, 1], f32, name="warm", tag="warm", bufs=1)
            nc.vector.memset(warm[:], 0.0)
            nc.scalar.activation(warm[:], warm[:],
                                 silu if act == "silu" else sigmoid)
            weights = (
                load_w(wgu_A, [KD, 2 * FSH], "(k p) f -> p k f", "wguA_sb", 2),
                load_w(wd_A, [MH, D], "(m p) d -> p m d", "wdA_sb", 1),
                load_w(wgu_B, [KD, 2 * FSH], "(k p) f -> p k f", "wguB_sb", 2),
                load_w(wd_B, [MH, D], "(m p) d -> p m d", "wdB_sb", 1),
                load_w(wgu_S, [KD, 2 * FSH], "(k p) f -> p k f", "wguS_sb", 2),
                load_w(wd_S, [MH, D], "(m p) d -> p m d", "wdS_sb", 1),
            )

            if repeat == 1:
                body(weights)
            elif flat:
                for _ in range(repeat):
                    body(weights)
            else:
                if unroll is None:
                    unroll = 4 if repeat % 4 == 0 else (2 if repeat % 2 == 0 else 1)
                with tc.For_i(0, repeat // unroll, 1,
                              hint_engines=(mybir.EngineType.PE,),
                              staggered_reset=True):
                    for _ in range(unroll):
                        body(weights)

    nc.compile()
    return nc


def prepare_pair(hidden_states, W_gate_router, expert_bias, Wg, Wu, Wd, Wg_s, Wu_s, Wd_s):
    """Pair-F-split sharding.  Experts sorted by count desc; pair k = (hot_k,
    cold_k) with hot = k-th largest, cold = k-th smallest.  Cores 2k / 2k+1
    hold F-half 0 / 1 of both experts in pair k (and of the shared expert,
    with shared token group g = k, exactly as the capacity design).
    Returns (nA, nB, in_maps, combine)."""
    x = np.ascontiguousarray(np.asarray(hidden_states, np.float32).reshape(T, D))
    sel, wts = _route(x, W_gate_router, expert_bias)

    tok = np.repeat(np.arange(T), TOP_K)
    expf = np.asarray(sel).reshape(-1)
    wf = np.asarray(wts).reshape(-1)
    counts = np.bincount(expf, minlength=E)
    order = np.argsort(expf, kind="stable")
    starts = np.zeros(E + 1, np.int64)
    starts[1:] = np.cumsum(counts)

    by_count = np.argsort(-counts, kind="stable")
    hot, cold = by_count[:E // 2], by_count[E // 2:][::-1]  # pair k: hot[k], cold[k]
    nA = int(counts[hot].max())
    nB = int(counts[cold].max())

    if MM_MODE == "f16":
        rnd = lambda a: np.ascontiguousarray(np.asarray(a, np.float32).astype(np.float16))
    else:
        rnd = lambda a: np.ascontiguousarray(a, np.float32)
    xrnd = rnd(x)
    Wg = np.asarray(Wg, np.float32)
    Wu = np.asarray(Wu, np.float32)
    Wd = np.asarray(Wd, np.float32)
    Wg_s = np.asarray(Wg_s, np.float32)
    Wu_s = np.asarray(Wu_s, np.float32)
    Wd_s = np.asarray(Wd_s, np.float32)

    ntot = nA + nB + SGRP
    in_maps = [None] * N_CORES
    pairinfo = []
    for k in range(E // 2):
        ea, eb = int(hot[k]), int(cold[k])
        sla = order[starts[ea]:starts[ea + 1]]
        slb = order[starts[eb]:starts[eb + 1]]
        na_k, nb_k = len(sla), len(slb)
        xa = np.zeros((ntot, D), xrnd.dtype)
        xa[:na_k] = xrnd[tok[sla]]
        xa[nA:nA + nb_k] = xrnd[tok[slb]]
        xa[nA + nB:] = xrnd[k * SGRP:(k + 1) * SGRP]
        xT = np.ascontiguousarray(xa.T)
        pairinfo.append((tok[sla], wf[sla].astype(np.float32),
                         tok[slb], wf[slb].astype(np.float32)))
        for hh in range(2):
            fs = slice(hh * FSH, (hh + 1) * FSH)
            in_maps[2 * k + hh] = {
                "x_in": xT,
                "wgu_A": rnd(np.concatenate([Wg[ea][:, fs], Wu[ea][:, fs]], axis=1)),
                "wd_A": rnd(Wd[ea][fs, :]),
                "wgu_B": rnd(np.concatenate([Wg[eb][:, fs], Wu[eb][:, fs]], axis=1)),
                "wd_B": rnd(Wd[eb][fs, :]),
                "wgu_S": rnd(np.concatenate([Wg_s[:, fs], Wu_s[:, fs]], axis=1)),
                "wd_S": rnd(Wd_s[fs, :]),
            }

    def combine(results):
        out = np.zeros((T, D), np.float32)
        for k in range(E // 2):
            toka, wa, tokb, wb = pairinfo[k]
            o = (results[2 * k]["o_out"].astype(np.float32)
                 + results[2 * k + 1]["o_out"].astype(np.float32)).T  # [ntot, D]
            out[toka] += o[:len(toka)] * wa[:, None]
            out[tokb] += o[nA:nA + len(tokb)] * wb[:, None]
            out[k * SGRP:(k + 1) * SGRP] += o[nA + nB:]
        return out.reshape(B, S, D)

    return nA, nB, in_maps, combine


def prepare_bal(hidden_states, W_gate_router, expert_bias, Wg, Wu, Wd, Wg_s, Wu_s, Wd_s):
    """Balanced sharding: core c gets count[c] routed slots (its expert) plus
    (ntot - count[c]) shared-expert tokens; every core runs exactly ntot
    identical-cost columns. Returns (splits, ntot, in_maps, combine)."""
    x = np.ascontiguousarray(np.asarray(hidden_states, np.float32).reshape(T, D))
    sel, wts = _route(x, W_gate_router, expert_bias)

    tok = np.repeat(np.arange(T), TOP_K)
    expf = np.asarray(sel).reshape(-1)
    wf = np.asarray(wts).reshape(-1)
    counts = np.bincount(expf, minlength=E)
    # ntot: smallest multiple of 8 >= (T*K + T)/8 that fits every expert and
    # leaves nonneg shared quotas.
    ntot = max(384, int(math.ceil((counts.max() + 1) / 8)) * 8)
    while N_CORES * ntot < T * TOP_K + T:
        ntot += 8

    order = np.argsort(expf, kind="stable")
    starts = np.zeros(E + 1, np.int64)
    starts[1:] = np.cumsum(counts)

    if MM_MODE == "f16":
        rnd = lambda a: np.ascontiguousarray(np.asarray(a, np.float32).astype(np.float16))
    else:
        rnd = lambda a: np.ascontiguousarray(a, np.float32)
    xrnd = rnd(x)
    Wg = np.asarray(Wg, np.float32)
    Wu = np.asarray(Wu, np.float32)
    Wd = np.asarray(Wd, np.float32)
    wgu_s = rnd(np.concatenate([np.asarray(Wg_s, np.float32),
                                np.asarray(Wu_s, np.float32)], axis=1))
    wd_s = rnd(np.asarray(Wd_s, np.float32))

    # shared-token quotas (fill each core to ntot)
    quotas = ntot - counts
    assert quotas.min() >= 0 and quotas.sum() >= T
    # trim surplus quota (if 8*ntot > N+T) off the cores with most quota
    surplus = int(quotas.sum() - T)
    qs = quotas.copy()
    while surplus > 0:
        i = int(np.argmax(qs))
        d = min(surplus, 8)
        qs[i] -= d
        surplus -= d
    bounds = np.zeros(E + 1, np.int64)
    bounds[1:] = np.cumsum(qs)

    in_maps, toklists, wlists, shlists, splits = [], [], [], [], []
    for c in range(N_CORES):
        sl = order[starts[c]:starts[c + 1]]
        n_c = int(counts[c])
        sh = np.arange(bounds[c], bounds[c + 1])  # shared tokens for core c
        xa = np.zeros((ntot, D), xrnd.dtype)
        xa[:n_c] = xrnd[tok[sl]]
        xa[n_c:n_c + len(sh)] = xrnd[sh]
        toklists.append(tok[sl])
        wlists.append(wf[sl].astype(np.float32))
        shlists.append(sh)
        splits.append(n_c)
        in_maps.append({
            "x_in": np.ascontiguousarray(xa.T),
            "wgu_e": rnd(np.concatenate([Wg[c], Wu[c]], axis=1)),
            "wd_e": rnd(Wd[c]),
            "wgu_s": wgu_s,
            "wd_s": wd_s,
        })

    def combine(results):
        out = np.zeros((T, D), np.float32)
        for c in range(N_CORES):
            o = results[c]["o_out"].astype(np.float32).T  # [ntot, D]
            n_c = splits[c]
            if n_c:
                np.add.at(out, toklists[c], o[:n_c] * wlists[c][:, None])
            sh = shlists[c]
            if len(sh):
                out[sh] += o[n_c:n_c + len(sh)]
        return out.reshape(B, S, D)

    return splits, ntot, in_maps, combine


def kernel_bal(hidden_states, W_gate_router, expert_bias, Wg, Wu, Wd, Wg_s, Wu_s, Wd_s):
    import jax
    from concourse.bass_utils import run_bass_kernel_spmd

    splits, ntot, in_maps, combine = prepare_bal(
        hidden_states, W_gate_router, expert_bias, Wg, Wu, Wd, Wg_s, Wu_s, Wd_s)
    results = []
    for c in range(N_CORES):
        key = ("bal", splits[c], ntot)
        nc = _compiled.get(key)
        if nc is None:
            nc = build_nc_bal(splits[c], ntot)
            _compiled[key] = nc
        res = run_bass_kernel_spmd(nc, [in_maps[c]], core_ids=[0])
        results.append(res.results[0])
    out = combine(results)
    return out.astype(np.asarray(hidden_states).dtype)


def kernel_pair(hidden_states, W_gate_router, expert_bias, Wg, Wu, Wd, Wg_s, Wu_s, Wd_s):
    """Pair-F-split path: uniform SPMD program, shard_map execution."""
    from concourse.bass_utils import run_bass_kernel_spmd

    nA, nB, in_maps, combine = prepare_pair(hidden_states, W_gate_router, expert_bias,
                                            Wg, Wu, Wd, Wg_s, Wu_s, Wd_s)
    key = ("pair", nA, nB)
    nc = _compiled.get(key)
    if nc is None:
        nc = build_nc_pair(nA, nB)
        _compiled[key] = nc
    res = run_bass_kernel_spmd(nc, in_maps, core_ids=list(range(N_CORES)))
    out = combine(res.results)
    return out.astype(np.asarray(hidden_states).dtype)


def kernel(hidden_states, W_gate_router, expert_bias, Wg, Wu, Wd, Wg_s, Wu_s, Wd_s):
    """Primary path: homogeneous SPMD, capacity-padded expert parallel."""
    from concourse.bass_utils import run_bass_kernel_spmd

    cap, in_maps, combine = prepare(hidden_states, W_gate_router, expert_bias,
                                    Wg, Wu, Wd, Wg_s, Wu_s, Wd_s)
    nc = _compiled.get(cap)
    if nc is None:
        nc = build_nc(cap)
        _compiled[cap] = nc
    res = run_bass_kernel_spmd(nc, in_maps, core_ids=list(range(N_CORES)))
    out = combine(res.results)
    return out.astype(np.asarray(hidden_states).dtype)

